# revision 33
# baseline (speedup 1.0000x reference)
"""Euler-tour connected-filter kernel for TRN2 (8 cores, data-parallel).

Math: v[i] = levels[root] + sum over root->i path of sigma_j * delta_j.
Place +sigma*delta at the tour slot where a node is entered and
-sigma*delta where it is exited; v[i] is then the inclusive prefix sum of
that 2N-long sequence at entry(i).  The whole 32-deep level-by-level
propagation collapses into one per-partition scan plus a 128-wide
cross-partition carry (triangular matmul).

Key packing trick: store lev[node] at entry slots and lev[parent] at exit
slots ("levseq").  Then for EVERY tour slot t:
    e[t] = sigma(attr_tour[t]) * (levseq[t] - levseq[t-1])
which is +sigma*delta at entries and exactly -sigma*delta at exits (the
subtraction is the exact IEEE negation, so closed subtrees cancel to the
rounding of the running sum).  The device therefore needs only TWO tour
arrays.  levseq is shipped as a [128, 4097] sliding view so the t-1 shift
never crosses a partition boundary; the virtual levseq[-1] is 0 and
attr_tour[0]=2.0 makes sigma=1 exactly, so slot 0 contributes
levels[root] like the reference's root override.

Host work is index arithmetic / layout only (depths, subtree sizes, tour
positions, sorting, gathers); every float op of the reference runs on
device.

Pixel phase: per core 524288 pixels sorted by source tour position; per
partition a contiguous window of the (f16) prefix array is fetched by
indirect DMA, run-start values are placed by gpsimd local_scatter, the
run mask is derived on device as (pb == 0) (real prefix values are never
0.0 since v >= levels[root] > 0.1), and a masked f16 scan expands runs to
per-pixel values; host unpermutes.  Output is f16 (max quantization 2^-11,
vs the 2e-2 correctness gate); measured end-to-end rel err 5.1e-4.

Measured on 8 axon-tunneled TRN2 cores (all 8 running concurrently):
~33us sustained on-device time per call, via interleaved marginal cost of
in-NEFF repetitions.  HBM traffic is ~7.25MB/core/call (attr_tour 2MB +
levseq 2MB + sidx 0.63MB + prefix write 1MB + window read 0.55MB + y 1MB),
i.e. ~220GB/s/core sustained under 8-core contention -- memory-bound, as
the target regime intends.  The earlier BFS level-expand formulation moved
~18MB/core through a 32-level serial chain and needed 10.3MB of inputs.
"""
import numpy as np

P = 128
N = 262144
TWO_N = 2 * N
TOUR_F = TWO_N // P          # 4096
PIX_PER_CORE = 524288
PIX_F = PIX_PER_CORE // P    # 4096
SEG = 2046                   # local_scatter num_elems limit (int16 units)
T = 4


# ======================= host: tour construction =======================

def build_tour(par):
    """entry/exit tour positions for one tree (children in node-id order)."""
    par = par.astype(np.int64)
    # depth via pointer doubling
    anc = par.copy(); anc[0] = N
    dep = np.ones(N, np.int64); dep[0] = 0
    anc_e = np.concatenate([anc, [N]])
    dep_e = np.concatenate([dep, [0]])
    while (anc_e[:N] != N).any():
        dep_e = dep_e + dep_e[anc_e]
        anc_e = anc_e[anc_e]
    depth = dep_e[:N]
    D = int(depth.max())
    order_by_depth = np.argsort(depth, kind="stable")
    counts = np.bincount(depth, minlength=D + 1)
    splits = np.split(order_by_depth, np.cumsum(counts)[:-1])

    # subtree sizes, deepest level first
    size = np.ones(N, np.int64)
    for dd in range(D, 0, -1):
        nd = splits[dd]
        np.add.at(size, par[nd], size[nd])

    # within-parent exclusive cumsum of sibling subtree sizes
    ch_order = np.argsort(par[1:], kind="stable") + 1
    pp = par[ch_order]
    sz = size[ch_order]
    cs = np.cumsum(sz) - sz
    starts = np.concatenate([[True], pp[1:] != pp[:-1]])
    start_cs = np.maximum.accumulate(np.where(starts, cs, -1))
    childoff = np.empty(N, np.int64)
    childoff[ch_order] = cs - start_cs
    childoff[0] = 0

    entry = np.zeros(N, np.int64)
    for dd in range(1, D + 1):
        nd = splits[dd]
        entry[nd] = entry[par[nd]] + 1 + 2 * childoff[nd]
    exit_ = entry + 2 * size - 1
    return entry, exit_


def build_tree_tensors(attr_t, lev_t, par, entry, exit_):
    """attr_tour [P, TOUR_F] and levseq [P, TOUR_F+1] device inputs."""
    attr_tour = np.empty(TWO_N, np.float32)
    levflat = np.empty(TWO_N, np.float32)
    attr_tour[entry] = attr_t
    attr_tour[exit_] = attr_t
    levflat[entry] = lev_t
    levflat[exit_] = lev_t[par.astype(np.int64)]
    attr_tour[0] = 2.0   # root: sigma(1000*(2-thr)) == 1.0 exactly
    arr2 = np.concatenate([np.zeros(1, np.float32), levflat])
    levseq = np.lib.stride_tricks.sliding_window_view(
        arr2, TOUR_F + 1)[::TOUR_F].copy()
    return attr_tour.reshape(P, TOUR_F).copy(), levseq


# ======================= host: pixel metadata =======================

def build_pixel_meta(srcpos_sorted_by_core):
    """Uniform (across 8 cores) window/segment layout + per-core scatter
    indices and masks.  srcpos_sorted_by_core: 8 arrays [PIX_PER_CORE]."""
    sp = [s.reshape(P, PIX_F) for s in srcpos_sorted_by_core]
    nlo = [s[:, 0].astype(np.int32) for s in sp]
    span = max(int((s[:, -1] - s[:, 0]).max()) for s in sp)

    runs = []
    for s, lo in zip(sp, nlo):
        per = []
        for p in range(P):
            row = s[p]
            st = np.flatnonzero(np.concatenate([[True], row[1:] != row[:-1]]))
            per.append(((row[st] - lo[p]).astype(np.int64), st.astype(np.int64)))
        runs.append(per)

    # prefix values are f16 in the window, so one scatter index per value
    # (no int16-pair splitting); dst segments over the 4096 pixel slots
    segs = []
    s0 = 0
    while s0 < PIX_F:
        w = min(SEG, PIX_F - s0)
        w -= w % 2
        segs.append((s0, w))
        s0 += w

    seg_meta = []
    for (s0, w) in segs:
        f0, f1 = s0, s0 + w
        w0g, w1g = 1 << 30, 0
        sel = []
        for per in runs:
            selc = []
            for p in range(P):
                m, ell = per[p]
                k = (ell >= f0) & (ell < f1)
                mm, ee = m[k], ell[k]
                selc.append((mm, ee))
                if mm.size:
                    w0g = min(w0g, int(mm.min()))
                    w1g = max(w1g, int(mm.max()) + 1)
            sel.append(selc)
        if w0g >= w1g:
            w0g, w1g = 0, 2
        w1g += (w1g - w0g) % 2   # even num_idxs
        seg_meta.append(dict(s0=s0, w=w, w0=w0g, w1=w1g, sel=sel))

    rlp = max(span + 2, max(sm["w1"] for sm in seg_meta) + 2)
    SPW = sum(sm["w1"] - sm["w0"] for sm in seg_meta)

    # no mask tensor: the device derives it as (pb == 0) — local_scatter
    # zeroes unwritten slots and real prefix values are never 0.0
    cores = []
    for ci in range(8):
        parts = []
        for sm in seg_meta:
            s0, w0, w1 = sm["s0"], sm["w0"], sm["w1"]
            idx = np.full((P, w1 - w0), -1, np.int16)
            for p in range(P):
                mm, ee = sm["sel"][ci][p]
                idx[p, mm - w0] = (ee - s0).astype(np.int16)
            parts.append(idx)
        cores.append(dict(sidx=np.concatenate(parts, axis=1),
                          roff=nlo[ci].reshape(P, 1).astype(np.int32)))
    return dict(rlp=rlp, SPW=SPW,
                segs=[(sm["s0"], sm["w"], sm["w0"], sm["w1"])
                      for sm in seg_meta],
                cores=cores)


# ======================= host: chunked pixel metadata =======================

def build_pixel_meta_chunked(srcpos_by_tree):
    """Pixels assigned to partition p = srcpos // TOUR_F (their tour chunk),
    so the scatter source is the scan output t_ps[p, :] directly in SBUF --
    no DRAM round-trip of the prefix, no indirect window gather.  Each core
    takes half of every chunk's (sorted) pixels; rows are padded to the max
    per-partition count C.  Returns per-core sidx plus the pixel-id lists
    needed to unpermute on the host."""
    percore = []   # 8 entries: list of P arrays of pixel ids (sorted)
    for t in range(T):
        srcpos = srcpos_by_tree[t]
        ordx = np.argsort(srcpos, kind="stable")
        ssp = srcpos[ordx]
        bounds = np.searchsorted(ssp, np.arange(P + 1) * TOUR_F)
        rows0, rows1 = [], []
        for p in range(P):
            lo, hi = int(bounds[p]), int(bounds[p + 1])
            mid = lo + ((hi - lo) + 1) // 2
            rows0.append(ordx[lo:mid])
            rows1.append(ordx[mid:hi])
        percore.append(rows0)
        percore.append(rows1)

    maxn = max(max(r.size for r in rows) for rows in percore)
    CG = maxn + 128   # placement target scale; C is finalized below

    # runs per (core, partition): m = srcpos - p*TOUR_F.  Run start slots are
    # placed PROPORTIONALLY (ell ~ m*CG/TOUR_F, pushed right on collision via
    # a running-max recurrence) so a destination segment always maps to a
    # narrow source window regardless of per-row pixel-count variation.
    # Gap slots between runs inherit the previous run's value in the scan
    # (pb==0 -> mask 1) and are simply never read back by the host.
    runs = []      # per core: per partition (m_run, ell_run)
    ells = []      # per core: per partition ell_of_pixel
    C = 0
    for ci in range(8):
        t = ci // 2
        srcpos = srcpos_by_tree[t]
        per, pere = [], []
        for p in range(P):
            row = srcpos[percore[ci][p]]
            n = row.size
            if n == 0:
                per.append((np.zeros(0, np.int64), np.zeros(0, np.int64)))
                pere.append(np.zeros(0, np.int64))
                continue
            st = np.flatnonzero(np.concatenate([[True], row[1:] != row[:-1]]))
            m = (row[st] - p * TOUR_F).astype(np.int64)
            rl = np.diff(np.concatenate([st, [n]]))
            cumprev = np.concatenate([[0], np.cumsum(rl)[:-1]])
            target = (m * CG) // TOUR_F
            b = np.maximum.accumulate(target - cumprev)
            ell = b + cumprev
            C = max(C, int(ell[-1] + rl[-1]))
            per.append((m, ell))
            pere.append(np.repeat(ell - cumprev, rl) + np.arange(n))
        runs.append(per)
        ells.append(pere)
    C += C % 2

    segs = []
    s0 = 0
    while s0 < C:
        w = min(SEG, C - s0)
        w -= w % 2
        segs.append((s0, w))
        s0 += w

    seg_meta = []
    for (s0, w) in segs:
        f0, f1 = s0, s0 + w
        w0g, w1g = 1 << 30, 0
        for per in runs:
            for p in range(P):
                m, ell = per[p]
                k = (ell >= f0) & (ell < f1)
                if k.any():
                    mm = m[k]
                    w0g = min(w0g, int(mm.min()))
                    w1g = max(w1g, int(mm.max()) + 1)
        if w0g >= w1g:
            w0g, w1g = 0, 2
        w1g = min(TOUR_F, w1g + (w1g - w0g) % 2)
        if (w1g - w0g) % 2:
            w0g -= 1
        seg_meta.append(dict(s0=s0, w=w, w0=w0g, w1=w1g))

    SPW = sum(sm["w1"] - sm["w0"] for sm in seg_meta)
    cores = []
    for ci in range(8):
        parts = []
        for sm in seg_meta:
            s0, w0, w1 = sm["s0"], sm["w0"], sm["w1"]
            f0, f1 = s0, s0 + sm["w"]
            idx = np.full((P, w1 - w0), -1, np.int16)
            for p in range(P):
                m, ell = runs[ci][p]
                k = (ell >= f0) & (ell < f1)
                idx[p, m[k] - w0] = (ell[k] - s0).astype(np.int16)
            parts.append(idx)
        cores.append(dict(sidx=np.concatenate(parts, axis=1)))
    return dict(C=C, SPW=SPW,
                segs=[(sm["s0"], sm["w"], sm["w0"], sm["w1"])
                      for sm in seg_meta],
                cores=cores, rows=percore)


# ======================= device program =======================
import sys
if '/opt/trn_rl_repo' not in sys.path:
    sys.path.insert(0, '/opt/trn_rl_repo')
from concourse import bass, mybir, tile, bacc
from concourse.bass_utils import run_bass_kernel_spmd

F32 = mybir.dt.float32
F16 = mybir.dt.float16
I32 = mybir.dt.int32
I16 = mybir.dt.int16


def build_bass_chunked(pixc, reps=1):
    """Chunk-partitioned pixel phase: scatter straight from the scan output
    in SBUF; no prefix DRAM round-trip, no indirect DMA anywhere."""
    C = pixc["C"]; SPW = pixc["SPW"]; segs = pixc["segs"]

    nc = bacc.Bacc(None, target_bir_lowering=False, debug=False)
    d_attr = nc.dram_tensor("attr_tour", [P, TOUR_F], F32, kind="ExternalInput")
    d_lseq = nc.dram_tensor("levseq", [P, TOUR_F + 1], F32, kind="ExternalInput")
    d_thr = nc.dram_tensor("thr", [1, 1], F32, kind="ExternalInput")
    d_tri = nc.dram_tensor("tri", [P, P], F32, kind="ExternalInput")
    d_sidx = nc.dram_tensor("sidx", [P, SPW], I16, kind="ExternalInput")
    d_y = nc.dram_tensor("y", [P, C], F16, kind="ExternalOutput")

    with tile.TileContext(nc) as tc:
        dbufs = 2 if reps > 1 else 1
        with tc.tile_pool(name="persist", bufs=1) as pp, \
             tc.tile_pool(name="work", bufs=1) as wp, \
             tc.tile_pool(name="io", bufs=dbufs) as iop, \
             tc.tile_pool(name="psum", bufs=dbufs, space="PSUM") as sp:
            t_ones = pp.tile([P, TOUR_F], F16)
            nc.vector.memset(t_ones[:], 1.0)
            t_tri = pp.tile([P, P], F32)
            nc.sync.dma_start(out=t_tri[:], in_=d_tri[:])
            t_thr = pp.tile([P, 1], F32)
            nc.sync.dma_start(out=t_thr[:], in_=d_thr[:].to_broadcast([P, 1]))
            t_thrb = pp.tile([P, 1], F32)
            nc.vector.tensor_scalar_mul(t_thrb[:], t_thr[:], -1000.0)

            for r in range(reps):
                t_attr = iop.tile([P, TOUR_F], F32, tag="attr")
                nc.sync.dma_start(out=t_attr[:], in_=d_attr[:])
                t_lseq = iop.tile([P, TOUR_F + 1], F32, tag="lseq")
                nc.sync.dma_start(out=t_lseq[:], in_=d_lseq[:])
                t_sidx = iop.tile([P, SPW], I16, tag="sidx")
                nc.sync.dma_start(out=t_sidx[:], in_=d_sidx[:])

                nc.scalar.activation(
                    out=t_attr[:], in_=t_attr[:],
                    func=mybir.ActivationFunctionType.Sigmoid,
                    bias=t_thrb[:, :1], scale=1000.0)
                t_e = wp.tile([P, TOUR_F], F32, tag="e")
                nc.vector.tensor_sub(out=t_e[:], in0=t_lseq[:, 1:TOUR_F + 1],
                                     in1=t_lseq[:, 0:TOUR_F])
                t_tot = wp.tile([P, 1], F32, tag="tot")
                nc.vector.scalar_tensor_tensor(
                    out=t_e[:], in0=t_e[:], scalar=0.0, in1=t_attr[:],
                    op0=mybir.AluOpType.bypass, op1=mybir.AluOpType.mult,
                    accum_out=t_tot[:])
                t_cpsum = sp.tile([P, 1], F32, tag="carry")
                nc.tensor.matmul(t_cpsum[:], t_tri[:], t_tot[:])
                t_carry = wp.tile([P, 1], F32, tag="carrys")
                nc.scalar.copy(out=t_carry[:], in_=t_cpsum[:])
                t_ps = wp.tile([P, TOUR_F], F16, tag="ps")
                nc.vector.tensor_tensor_scan(
                    out=t_ps[:], data0=t_ones[:], data1=t_e[:],
                    initial=t_carry[:, :1],
                    op0=mybir.AluOpType.mult, op1=mybir.AluOpType.add)

                # pixel phase straight from SBUF
                t_pb = wp.tile([P, C], I16, tag="pb")
                col = 0
                for (s0, w, w0, w1) in segs:
                    nw = w1 - w0
                    nc.gpsimd.local_scatter(
                        out_ap=t_pb[:, s0:s0 + w],
                        data_ap=t_ps[:, w0:w1].bitcast(I16),
                        idxs_ap=t_sidx[:, col:col + nw],
                        channels=P, num_elems=w, num_idxs=nw)
                    col += nw
                t_am = wp.tile([P, C], F16, tag="am")
                nc.vector.tensor_single_scalar(
                    out=t_am[:], in_=t_pb[:].bitcast(F16), scalar=0.0,
                    op=mybir.AluOpType.is_equal)
                t_y = wp.tile([P, C], F16, tag="y")
                nc.vector.tensor_tensor_scan(
                    out=t_y[:], data0=t_am[:], data1=t_pb[:].bitcast(F16),
                    initial=0.0, op0=mybir.AluOpType.mult,
                    op1=mybir.AluOpType.add)
                nc.sync.dma_start(out=d_y[:], in_=t_y[:])
    nc.finalize()
    return nc


def build_bass(pix, reps=1, partial_write=False):
    rlp = pix["rlp"]; SPW = pix["SPW"]; segs = pix["segs"]; NW = pix["NW"]

    nc = bacc.Bacc(None, target_bir_lowering=False, debug=False)
    d_attr = nc.dram_tensor("attr_tour", [P, TOUR_F], F32, kind="ExternalInput")
    d_lseq = nc.dram_tensor("levseq", [P, TOUR_F + 1], F32, kind="ExternalInput")
    d_thr = nc.dram_tensor("thr", [1, 1], F32, kind="ExternalInput")
    d_tri = nc.dram_tensor("tri", [P, P], F32, kind="ExternalInput")
    d_roff = nc.dram_tensor("roff", [P, 1], I32, kind="ExternalInput")
    d_woff = nc.dram_tensor("woff", [P, 1], I32, kind="ExternalInput")
    d_sidx = nc.dram_tensor("sidx", [P, SPW], I16, kind="ExternalInput")
    # f16 output: max rel quantization 2^-11, far under the 2e-2 gate;
    # halves the output write + host transfer
    d_y = nc.dram_tensor("y", [P, PIX_F], F16, kind="ExternalOutput")

    TAILF = (rlp + P - 1) // P + 1
    VNF = TOUR_F + TAILF

    with tile.TileContext(nc) as tc:
        dbufs = 2 if reps > 1 else 1
        with tc.tile_pool(name="dram", bufs=1, space="DRAM") as dpool, \
             tc.tile_pool(name="persist", bufs=1) as pp, \
             tc.tile_pool(name="work", bufs=dbufs) as wp, \
             tc.tile_pool(name="io", bufs=dbufs) as iop, \
             tc.tile_pool(name="psum", bufs=dbufs, space="PSUM") as sp:
            vflat = dpool.tile([P * VNF, 1], F16)

            # persistent constants
            t_ones = pp.tile([P, TOUR_F], F32)
            nc.vector.memset(t_ones[:], 1.0)
            t_tri = pp.tile([P, P], F32)
            nc.sync.dma_start(out=t_tri[:], in_=d_tri[:])
            t_thr = pp.tile([P, 1], F32)
            nc.sync.dma_start(out=t_thr[:], in_=d_thr[:].to_broadcast([P, 1]))
            t_thrb = pp.tile([P, 1], F32)
            nc.vector.tensor_scalar_mul(t_thrb[:], t_thr[:], -1000.0)
            if partial_write:
                t_woff = pp.tile([P, 1], I32)
                nc.sync.dma_start(out=t_woff[:], in_=d_woff[:])
            # zero-fill the window-overhang tail past position 2N
            t_tz = pp.tile([P, TAILF], F16)
            nc.vector.memset(t_tz[:], 0.0)
            nc.sync.dma_start(out=vflat[TWO_N:P * VNF, 0:1], in_=t_tz[:])

            for r in range(reps):
                t_attr = iop.tile([P, TOUR_F], F32, tag="attr")
                nc.sync.dma_start(out=t_attr[:], in_=d_attr[:])
                t_lseq = iop.tile([P, TOUR_F + 1], F32, tag="lseq")
                nc.sync.dma_start(out=t_lseq[:], in_=d_lseq[:])

                # sigma = sigmoid(1000*attr - 1000*thr)   (unclamped; the
                # +-12 clamp only changes sigma by <7e-6)
                nc.scalar.activation(
                    out=t_attr[:], in_=t_attr[:],
                    func=mybir.ActivationFunctionType.Sigmoid,
                    bias=t_thrb[:, :1], scale=1000.0)

                # e = sigma * (levseq[t] - levseq[t-1]); totals = row sums
                t_e = wp.tile([P, TOUR_F], F32, tag="e")
                nc.vector.tensor_sub(out=t_e[:], in0=t_lseq[:, 1:TOUR_F + 1],
                                     in1=t_lseq[:, 0:TOUR_F])
                t_tot = wp.tile([P, 1], F32, tag="tot")
                nc.vector.scalar_tensor_tensor(
                    out=t_e[:], in0=t_e[:], scalar=0.0, in1=t_attr[:],
                    op0=mybir.AluOpType.bypass, op1=mybir.AluOpType.mult,
                    accum_out=t_tot[:])

                # cross-partition exclusive prefix of totals (strict lower
                # triangular ones matmul), used as the scan's initial state
                t_cpsum = sp.tile([P, 1], F32, tag="carry")
                nc.tensor.matmul(t_cpsum[:], t_tri[:], t_tot[:])
                t_carry = wp.tile([P, 1], F32, tag="carrys")
                nc.scalar.copy(out=t_carry[:], in_=t_cpsum[:])

                t_ps = wp.tile([P, TOUR_F], F16, tag="ps")
                nc.vector.tensor_tensor_scan(
                    out=t_ps[:], data0=t_ones[:], data1=t_e[:],
                    initial=t_carry[:, :1],
                    op0=mybir.AluOpType.mult, op1=mybir.AluOpType.add)
                if partial_write:
                    # only the chunks this core's pixel windows read
                    # (permuted into partitions [0, NW)) take the round-trip
                    nc.gpsimd.indirect_dma_start(
                        out=vflat[:], out_offset=bass.IndirectOffsetOnAxis(
                            ap=t_woff[0:NW, 0:1], axis=0),
                        in_=t_ps[0:NW, :], in_offset=None)
                else:
                    nc.sync.dma_start(out=vflat[0:TWO_N, 0:1], in_=t_ps[:])

                # ---- pixel phase ----
                t_roff = wp.tile([P, 1], I32, tag="roff")
                nc.sync.dma_start(out=t_roff[:], in_=d_roff[:])
                t_sidx = iop.tile([P, SPW], I16, tag="sidx")
                nc.sync.dma_start(out=t_sidx[:], in_=d_sidx[:])

                t_pr = wp.tile([P, rlp], F16, tag="pr")
                nc.gpsimd.indirect_dma_start(
                    out=t_pr[:], out_offset=None, in_=vflat[:],
                    in_offset=bass.IndirectOffsetOnAxis(ap=t_roff[:, 0:1],
                                                        axis=0))
                t_pb = wp.tile([P, PIX_F], I16, tag="pb")
                col = 0
                for (s0, w, w0, w1) in segs:
                    nw = w1 - w0
                    nc.gpsimd.local_scatter(
                        out_ap=t_pb[:, s0:s0 + w],
                        data_ap=t_pr[:, w0:w1].bitcast(I16),
                        idxs_ap=t_sidx[:, col:col + nw],
                        channels=P, num_elems=w, num_idxs=nw)
                    col += nw
                # mask = (pb == 0): 1.0 inside runs (keep state), 0.0 at
                # run starts (reset to the scattered value)
                t_am = wp.tile([P, PIX_F], F16, tag="am")
                nc.vector.tensor_single_scalar(
                    out=t_am[:], in_=t_pb[:].bitcast(F16), scalar=0.0,
                    op=mybir.AluOpType.is_equal)
                t_y = wp.tile([P, PIX_F], F16, tag="y")
                nc.vector.tensor_tensor_scan(
                    out=t_y[:], data0=t_am[:], data1=t_pb[:].bitcast(F16),
                    initial=0.0, op0=mybir.AluOpType.mult,
                    op1=mybir.AluOpType.add)
                nc.sync.dma_start(out=d_y[:], in_=t_y[:])
    nc.finalize()
    return nc


# ======================= orchestration =======================

def build_all(attr, levels, parent, p2n):
    """All host-side metadata + per-core input maps (minus thr/tri)."""
    per_tree = []
    for t in range(T):
        entry, exit_ = build_tour(parent[t])
        at, ls = build_tree_tensors(attr[t], levels[t], parent[t], entry, exit_)
        per_tree.append(dict(entry=entry, attr_tour=at, levseq=ls))

    srcpos_by_core, my_by_core = [], []
    for t in range(T):
        srcpos = per_tree[t]["entry"][p2n[t].astype(np.int64)]
        ordx = np.argsort(srcpos, kind="stable")
        for half in range(2):
            my = ordx[half * PIX_PER_CORE:(half + 1) * PIX_PER_CORE]
            my_by_core.append(my)
            srcpos_by_core.append(srcpos[my])
    pix = build_pixel_meta(srcpos_by_core)
    pix["my"] = my_by_core
    pix["per_tree"] = per_tree

    # ---- chunk permutation: each core's pixel windows only touch ~half of
    # the tour, so only those prefix chunks need the DRAM round-trip.  The
    # write slice [0:NW) is a shared program immediate, so per core we
    # permute tour chunks across partitions to put the needed chunks first;
    # the carry matmul's triangular matrix is permuted to match and the
    # write destinations come from a per-core offset tensor. ----
    ranges = []
    for c in pix["cores"]:
        ro = c["roff"].ravel().astype(np.int64)
        q0 = int(ro.min()) // TOUR_F
        q1 = min(P, -(-(int(ro.max()) + pix["rlp"]) // TOUR_F))
        ranges.append((q0, q1))
    NW = max(q1 - q0 for (q0, q1) in ranges)
    pix["NW"] = NW
    for ci, c in enumerate(pix["cores"]):
        q0, q1 = ranges[ci]
        need = list(range(q0, q1))
        rest = [q for q in range(P) if q < q0 or q >= q1]
        pad = rest[:NW - len(need)]
        tail = rest[NW - len(need):]
        chunk = np.array(need + pad + tail, np.int64)   # chunk_of_partition
        assert chunk.size == P and np.array_equal(np.sort(chunk), np.arange(P))
        c["chunk"] = chunk
        c["woff"] = (chunk * TOUR_F).astype(np.int32).reshape(P, 1)
        # tri[k, m] = 1 iff chunk[k] < chunk[m]
        c["tri"] = (chunk[:, None] < chunk[None, :]).astype(np.float32)
    return pix


def make_in_maps(pix, thr, perm=False):
    """perm=True pairs with build_bass(partial_write=True): tour chunks are
    permuted per core so the needed prefix chunks sit in partitions [0,NW)."""
    thr2 = np.asarray(thr, np.float32).reshape(1, 1)
    ident = np.arange(P, dtype=np.int64)
    tri_std = (ident[:, None] < ident[None, :]).astype(np.float32)
    in_maps = []
    for ci in range(8):
        t = ci // 2
        c = pix["cores"][ci]
        chunk = c["chunk"] if perm else ident
        at = pix["per_tree"][t]["attr_tour"][chunk]
        ls = pix["per_tree"][t]["levseq"][chunk]
        in_maps.append(dict(
            attr_tour=np.ascontiguousarray(at),
            levseq=np.ascontiguousarray(ls),
            thr=thr2, tri=c["tri"] if perm else tri_std, roff=c["roff"],
            woff=(chunk * TOUR_F).astype(np.int32).reshape(P, 1),
            sidx=c["sidx"]))
    return in_maps


def build_all_chunked(attr, levels, parent, p2n):
    per_tree = []
    srcpos_by_tree = []
    for t in range(T):
        entry, exit_ = build_tour(parent[t])
        at, ls = build_tree_tensors(attr[t], levels[t], parent[t], entry, exit_)
        per_tree.append(dict(attr_tour=at, levseq=ls))
        srcpos_by_tree.append(entry[p2n[t].astype(np.int64)])
    pixc = build_pixel_meta_chunked(srcpos_by_tree)
    pixc["per_tree"] = per_tree
    return pixc


def make_in_maps_chunked(pixc, thr):
    thr2 = np.asarray(thr, np.float32).reshape(1, 1)
    ident = np.arange(P, dtype=np.int64)
    tri = (ident[:, None] < ident[None, :]).astype(np.float32)
    in_maps = []
    for ci in range(8):
        t = ci // 2
        in_maps.append(dict(
            attr_tour=pixc["per_tree"][t]["attr_tour"],
            levseq=pixc["per_tree"][t]["levseq"],
            thr=thr2, tri=tri, sidx=pixc["cores"][ci]["sidx"]))
    return in_maps


def kernel(**inputs):
    x = np.asarray(inputs["x"])
    attr = np.asarray(inputs["attr_norm"], dtype=np.float32)
    levels = np.asarray(inputs["levels"], dtype=np.float32)
    thr = np.asarray(inputs["thr"], dtype=np.float32)
    parent = np.asarray(inputs["parent"], dtype=np.int32)
    p2n = np.asarray(inputs["pixel_to_node"], dtype=np.int32)
    B, Cc, H, W = x.shape

    pixc = build_all_chunked(attr.reshape(T, -1), levels.reshape(T, -1),
                             parent.reshape(T, -1), p2n.reshape(T, -1))
    nc = build_bass_chunked(pixc)
    in_maps = make_in_maps_chunked(pixc, thr)
    res = run_bass_kernel_spmd(nc, in_maps, list(range(8)))

    y = np.zeros((T, H * W), np.float32)
    for ci in range(8):
        t = ci // 2
        yc = res.results[ci]["y"]
        for p in range(P):
            ids = pixc["rows"][ci][p]
            if ids.size:
                y[t][ids] = yc[p, :ids.size]
    return y.reshape(B, Cc, H, W)


# revision 40
# speedup vs baseline: 1.4915x; 1.4915x over previous
"""Euler-tour connected-filter kernel for TRN2 (8 cores, data-parallel).

Math: v[i] = levels[root] + sum over root->i path of sigma_j * delta_j.
Place +sigma*delta at the tour slot where a node is entered and
-sigma*delta where it is exited; v[i] is then the inclusive prefix sum of
that 2N-long sequence at entry(i).  The whole 32-deep level-by-level
propagation collapses into one per-partition scan plus a 128-wide
cross-partition carry (triangular matmul).

Key packing trick: store lev[node] at entry slots and lev[parent] at exit
slots ("levseq").  Then for EVERY tour slot t:
    e[t] = sigma(attr_tour[t]) * (levseq[t] - levseq[t-1])
which is +sigma*delta at entries and exactly -sigma*delta at exits (the
subtraction is the exact IEEE negation, so closed subtrees cancel to the
rounding of the running sum).  The device therefore needs only TWO tour
arrays.  levseq is shipped as a [128, 4097] sliding view so the t-1 shift
never crosses a partition boundary; the virtual levseq[-1] is 0 and
attr_tour[0]=2.0 makes sigma=1 exactly, so slot 0 contributes
levels[root] like the reference's root override.

Host work is index arithmetic / layout only (depths, subtree sizes, tour
positions, sorting, gathers); every float op of the reference runs on
device.

Pixel phase: per core 524288 pixels sorted by source tour position; per
partition a contiguous window of the (f16) prefix array is fetched by
indirect DMA, run-start values are placed by gpsimd local_scatter, the
run mask is derived on device as (pb == 0) (real prefix values are never
0.0 since v >= levels[root] > 0.1), and a masked f16 scan expands runs to
per-pixel values; host unpermutes.  Output is f16 (max quantization 2^-11,
vs the 2e-2 correctness gate); measured end-to-end rel err 5.1e-4.

Measured on 8 axon-tunneled TRN2 cores (all 8 running concurrently):
~33us sustained on-device time per call, via interleaved marginal cost of
in-NEFF repetitions.  HBM traffic is ~7.25MB/core/call (attr_tour 2MB +
levseq 2MB + sidx 0.63MB + prefix write 1MB + window read 0.55MB + y 1MB),
i.e. ~220GB/s/core sustained under 8-core contention -- memory-bound, as
the target regime intends.  The earlier BFS level-expand formulation moved
~18MB/core through a 32-level serial chain and needed 10.3MB of inputs.
"""
import numpy as np

P = 128
N = 262144
TWO_N = 2 * N
TOUR_F = TWO_N // P          # 4096
PIX_PER_CORE = 524288
PIX_F = PIX_PER_CORE // P    # 4096
SEG = 2046                   # local_scatter num_elems limit (int16 units)
T = 4


# ======================= host: tour construction =======================

def build_tour(par):
    """entry/exit tour positions for one tree (children in node-id order)."""
    par = par.astype(np.int64)
    # depth via pointer doubling
    anc = par.copy(); anc[0] = N
    dep = np.ones(N, np.int64); dep[0] = 0
    anc_e = np.concatenate([anc, [N]])
    dep_e = np.concatenate([dep, [0]])
    while (anc_e[:N] != N).any():
        dep_e = dep_e + dep_e[anc_e]
        anc_e = anc_e[anc_e]
    depth = dep_e[:N]
    D = int(depth.max())
    order_by_depth = np.argsort(depth, kind="stable")
    counts = np.bincount(depth, minlength=D + 1)
    splits = np.split(order_by_depth, np.cumsum(counts)[:-1])

    # subtree sizes, deepest level first
    size = np.ones(N, np.int64)
    for dd in range(D, 0, -1):
        nd = splits[dd]
        np.add.at(size, par[nd], size[nd])

    # within-parent exclusive cumsum of sibling subtree sizes
    ch_order = np.argsort(par[1:], kind="stable") + 1
    pp = par[ch_order]
    sz = size[ch_order]
    cs = np.cumsum(sz) - sz
    starts = np.concatenate([[True], pp[1:] != pp[:-1]])
    start_cs = np.maximum.accumulate(np.where(starts, cs, -1))
    childoff = np.empty(N, np.int64)
    childoff[ch_order] = cs - start_cs
    childoff[0] = 0

    entry = np.zeros(N, np.int64)
    for dd in range(1, D + 1):
        nd = splits[dd]
        entry[nd] = entry[par[nd]] + 1 + 2 * childoff[nd]
    exit_ = entry + 2 * size - 1
    return entry, exit_


def build_tree_tensors(attr_t, lev_t, par, entry, exit_):
    """attr_tour [P, TOUR_F] and levseq [P, TOUR_F+1] device inputs."""
    attr_tour = np.empty(TWO_N, np.float32)
    levflat = np.empty(TWO_N, np.float32)
    attr_tour[entry] = attr_t
    attr_tour[exit_] = attr_t
    levflat[entry] = lev_t
    levflat[exit_] = lev_t[par.astype(np.int64)]
    attr_tour[0] = 2.0   # root: sigma(1000*(2-thr)) == 1.0 exactly
    arr2 = np.concatenate([np.zeros(1, np.float32), levflat])
    levseq = np.lib.stride_tricks.sliding_window_view(
        arr2, TOUR_F + 1)[::TOUR_F].copy()
    return attr_tour.reshape(P, TOUR_F).copy(), levseq


# ======================= host: pixel metadata =======================

def build_pixel_meta(srcpos_sorted_by_core):
    """Uniform (across 8 cores) window/segment layout + per-core scatter
    indices and masks.  srcpos_sorted_by_core: 8 arrays [PIX_PER_CORE]."""
    sp = [s.reshape(P, PIX_F) for s in srcpos_sorted_by_core]
    nlo = [s[:, 0].astype(np.int32) for s in sp]
    span = max(int((s[:, -1] - s[:, 0]).max()) for s in sp)

    runs = []
    for s, lo in zip(sp, nlo):
        per = []
        for p in range(P):
            row = s[p]
            st = np.flatnonzero(np.concatenate([[True], row[1:] != row[:-1]]))
            per.append(((row[st] - lo[p]).astype(np.int64), st.astype(np.int64)))
        runs.append(per)

    # prefix values are f16 in the window, so one scatter index per value
    # (no int16-pair splitting); dst segments over the 4096 pixel slots
    segs = []
    s0 = 0
    while s0 < PIX_F:
        w = min(SEG, PIX_F - s0)
        w -= w % 2
        segs.append((s0, w))
        s0 += w

    seg_meta = []
    for (s0, w) in segs:
        f0, f1 = s0, s0 + w
        w0g, w1g = 1 << 30, 0
        sel = []
        for per in runs:
            selc = []
            for p in range(P):
                m, ell = per[p]
                k = (ell >= f0) & (ell < f1)
                mm, ee = m[k], ell[k]
                selc.append((mm, ee))
                if mm.size:
                    w0g = min(w0g, int(mm.min()))
                    w1g = max(w1g, int(mm.max()) + 1)
            sel.append(selc)
        if w0g >= w1g:
            w0g, w1g = 0, 2
        w1g += (w1g - w0g) % 2   # even num_idxs
        seg_meta.append(dict(s0=s0, w=w, w0=w0g, w1=w1g, sel=sel))

    rlp = max(span + 2, max(sm["w1"] for sm in seg_meta) + 2)
    SPW = sum(sm["w1"] - sm["w0"] for sm in seg_meta)

    # no mask tensor: the device derives it as (pb == 0) — local_scatter
    # zeroes unwritten slots and real prefix values are never 0.0
    cores = []
    for ci in range(8):
        parts = []
        for sm in seg_meta:
            s0, w0, w1 = sm["s0"], sm["w0"], sm["w1"]
            idx = np.full((P, w1 - w0), -1, np.int16)
            for p in range(P):
                mm, ee = sm["sel"][ci][p]
                idx[p, mm - w0] = (ee - s0).astype(np.int16)
            parts.append(idx)
        cores.append(dict(sidx=np.concatenate(parts, axis=1),
                          roff=nlo[ci].reshape(P, 1).astype(np.int32)))
    return dict(rlp=rlp, SPW=SPW,
                segs=[(sm["s0"], sm["w"], sm["w0"], sm["w1"])
                      for sm in seg_meta],
                cores=cores)


# ======================= host: chunked pixel metadata =======================

def build_pixel_meta_chunked(srcpos_by_tree):
    """Pixels assigned to partition p = srcpos // TOUR_F (their tour chunk),
    so the scatter source is the scan output t_ps[p, :] directly in SBUF --
    no DRAM round-trip of the prefix, no indirect window gather.  Each core
    takes half of every chunk's (sorted) pixels; rows are padded to the max
    per-partition count C.  Returns per-core sidx plus the pixel-id lists
    needed to unpermute on the host."""
    percore = []   # 8 entries: list of P arrays of pixel ids (sorted)
    for t in range(T):
        srcpos = srcpos_by_tree[t]
        ordx = np.argsort(srcpos, kind="stable")
        ssp = srcpos[ordx]
        bounds = np.searchsorted(ssp, np.arange(P + 1) * TOUR_F)
        rows0, rows1 = [], []
        for p in range(P):
            lo, hi = int(bounds[p]), int(bounds[p + 1])
            mid = lo + ((hi - lo) + 1) // 2
            rows0.append(ordx[lo:mid])
            rows1.append(ordx[mid:hi])
        percore.append(rows0)
        percore.append(rows1)

    maxn = max(max(r.size for r in rows) for rows in percore)
    CG = maxn + 128   # placement target scale; C is finalized below

    # runs per (core, partition): m = srcpos - p*TOUR_F.  Run start slots are
    # placed PROPORTIONALLY (ell ~ m*CG/TOUR_F, pushed right on collision via
    # a running-max recurrence) so a destination segment always maps to a
    # narrow source window regardless of per-row pixel-count variation.
    # Gap slots between runs inherit the previous run's value in the scan
    # (pb==0 -> mask 1) and are simply never read back by the host.
    runs = []      # per core: per partition (m_run, ell_run)
    ells = []      # per core: per partition ell_of_pixel
    C = 0
    for ci in range(8):
        t = ci // 2
        srcpos = srcpos_by_tree[t]
        per, pere = [], []
        for p in range(P):
            row = srcpos[percore[ci][p]]
            n = row.size
            if n == 0:
                per.append((np.zeros(0, np.int64), np.zeros(0, np.int64)))
                pere.append(np.zeros(0, np.int64))
                continue
            st = np.flatnonzero(np.concatenate([[True], row[1:] != row[:-1]]))
            m = (row[st] - p * TOUR_F).astype(np.int64)
            rl = np.diff(np.concatenate([st, [n]]))
            cumprev = np.concatenate([[0], np.cumsum(rl)[:-1]])
            # proportional target, capped so the packed remainder always
            # fits in CG slots (=> end <= CG for every row)
            target = np.minimum((m * CG) // TOUR_F, CG - (n - cumprev))
            b = np.maximum.accumulate(target - cumprev)
            ell = b + cumprev
            C = max(C, int(ell[-1] + rl[-1]))
            per.append((m, ell))
            pere.append(np.repeat(ell - cumprev, rl) + np.arange(n))
        runs.append(per)
        ells.append(pere)
    C += C % 2

    segs = []
    s0 = 0
    while s0 < C:
        w = min(SEG, C - s0)
        w -= w % 2
        segs.append((s0, w))
        s0 += w

    seg_meta = []
    for (s0, w) in segs:
        f0, f1 = s0, s0 + w
        w0g, w1g = 1 << 30, 0
        for per in runs:
            for p in range(P):
                m, ell = per[p]
                k = (ell >= f0) & (ell < f1)
                if k.any():
                    mm = m[k]
                    w0g = min(w0g, int(mm.min()))
                    w1g = max(w1g, int(mm.max()) + 1)
        if w0g >= w1g:
            w0g, w1g = 0, 2
        w1g = min(TOUR_F, w1g + (w1g - w0g) % 2)
        if (w1g - w0g) % 2:
            w0g -= 1
        seg_meta.append(dict(s0=s0, w=w, w0=w0g, w1=w1g))

    SPW = sum(sm["w1"] - sm["w0"] for sm in seg_meta)
    cores = []
    for ci in range(8):
        parts = []
        for sm in seg_meta:
            s0, w0, w1 = sm["s0"], sm["w0"], sm["w1"]
            f0, f1 = s0, s0 + sm["w"]
            idx = np.full((P, w1 - w0), -1, np.int16)
            for p in range(P):
                m, ell = runs[ci][p]
                k = (ell >= f0) & (ell < f1)
                idx[p, m[k] - w0] = (ell[k] - s0).astype(np.int16)
            parts.append(idx)
        cores.append(dict(sidx=np.concatenate(parts, axis=1)))
    return dict(C=C, SPW=SPW,
                segs=[(sm["s0"], sm["w"], sm["w0"], sm["w1"])
                      for sm in seg_meta],
                cores=cores, rows=percore, ells=ells)


# ======================= device program =======================
import sys
if '/opt/trn_rl_repo' not in sys.path:
    sys.path.insert(0, '/opt/trn_rl_repo')
from concourse import bass, mybir, tile, bacc
from concourse.bass_utils import run_bass_kernel_spmd

F32 = mybir.dt.float32
F16 = mybir.dt.float16
I32 = mybir.dt.int32
I16 = mybir.dt.int16


def build_bass_chunked(pixc, reps=1):
    """Chunk-partitioned pixel phase: scatter straight from the scan output
    in SBUF; no prefix DRAM round-trip, no indirect DMA anywhere."""
    C = pixc["C"]; SPW = pixc["SPW"]; segs = pixc["segs"]

    nc = bacc.Bacc(None, target_bir_lowering=False, debug=False)
    d_attr = nc.dram_tensor("attr_tour", [P, TOUR_F], F32, kind="ExternalInput")
    d_lseq = nc.dram_tensor("levseq", [P, TOUR_F + 1], F32, kind="ExternalInput")
    d_thr = nc.dram_tensor("thr", [1, 1], F32, kind="ExternalInput")
    d_tri = nc.dram_tensor("tri", [P, P], F32, kind="ExternalInput")
    d_sidx = nc.dram_tensor("sidx", [P, SPW], I16, kind="ExternalInput")
    d_y = nc.dram_tensor("y", [P, C], F16, kind="ExternalOutput")

    with tile.TileContext(nc) as tc:
        dbufs = 2 if reps > 1 else 1
        with tc.tile_pool(name="persist", bufs=1) as pp, \
             tc.tile_pool(name="work", bufs=1) as wp, \
             tc.tile_pool(name="hot", bufs=dbufs) as hp, \
             tc.tile_pool(name="io", bufs=dbufs) as iop, \
             tc.tile_pool(name="psum", bufs=dbufs, space="PSUM") as sp:
            t_ones = pp.tile([P, TOUR_F], F16)
            nc.vector.memset(t_ones[:], 1.0)
            t_tri = pp.tile([P, P], F32)
            nc.sync.dma_start(out=t_tri[:], in_=d_tri[:])
            t_thr = pp.tile([P, 1], F32)
            nc.sync.dma_start(out=t_thr[:], in_=d_thr[:].to_broadcast([P, 1]))
            t_thrb = pp.tile([P, 1], F32)
            nc.vector.tensor_scalar_mul(t_thrb[:], t_thr[:], -1000.0)

            for r in range(reps):
                t_attr = iop.tile([P, TOUR_F], F32, tag="attr")
                nc.sync.dma_start(out=t_attr[:], in_=d_attr[:])
                t_lseq = iop.tile([P, TOUR_F + 1], F32, tag="lseq")
                nc.sync.dma_start(out=t_lseq[:], in_=d_lseq[:])
                t_sidx = iop.tile([P, SPW], I16, tag="sidx")
                nc.sync.dma_start(out=t_sidx[:], in_=d_sidx[:])

                nc.scalar.activation(
                    out=t_attr[:], in_=t_attr[:],
                    func=mybir.ActivationFunctionType.Sigmoid,
                    bias=t_thrb[:, :1], scale=1000.0)
                t_e = wp.tile([P, TOUR_F], F32, tag="e")
                nc.vector.tensor_sub(out=t_e[:], in0=t_lseq[:, 1:TOUR_F + 1],
                                     in1=t_lseq[:, 0:TOUR_F])
                t_tot = wp.tile([P, 1], F32, tag="tot")
                nc.vector.scalar_tensor_tensor(
                    out=t_e[:], in0=t_e[:], scalar=0.0, in1=t_attr[:],
                    op0=mybir.AluOpType.bypass, op1=mybir.AluOpType.mult,
                    accum_out=t_tot[:])
                t_cpsum = sp.tile([P, 1], F32, tag="carry")
                nc.tensor.matmul(t_cpsum[:], t_tri[:], t_tot[:])
                t_carry = wp.tile([P, 1], F32, tag="carrys")
                nc.scalar.copy(out=t_carry[:], in_=t_cpsum[:])
                t_ps = hp.tile([P, TOUR_F], F16, tag="ps")
                nc.vector.tensor_tensor_scan(
                    out=t_ps[:], data0=t_ones[:], data1=t_e[:],
                    initial=t_carry[:, :1],
                    op0=mybir.AluOpType.mult, op1=mybir.AluOpType.add)

                # pixel phase straight from SBUF
                t_pb = wp.tile([P, C], I16, tag="pb")
                col = 0
                for (s0, w, w0, w1) in segs:
                    nw = w1 - w0
                    nc.gpsimd.local_scatter(
                        out_ap=t_pb[:, s0:s0 + w],
                        data_ap=t_ps[:, w0:w1].bitcast(I16),
                        idxs_ap=t_sidx[:, col:col + nw],
                        channels=P, num_elems=w, num_idxs=nw)
                    col += nw
                t_am = wp.tile([P, C], F16, tag="am")
                nc.vector.tensor_single_scalar(
                    out=t_am[:], in_=t_pb[:].bitcast(F16), scalar=0.0,
                    op=mybir.AluOpType.is_equal)
                t_y = hp.tile([P, C], F16, tag="y")
                nc.vector.tensor_tensor_scan(
                    out=t_y[:], data0=t_am[:], data1=t_pb[:].bitcast(F16),
                    initial=0.0, op0=mybir.AluOpType.mult,
                    op1=mybir.AluOpType.add)
                nc.sync.dma_start(out=d_y[:], in_=t_y[:])
    nc.finalize()
    return nc


def build_bass(pix, reps=1, partial_write=False):
    rlp = pix["rlp"]; SPW = pix["SPW"]; segs = pix["segs"]; NW = pix["NW"]

    nc = bacc.Bacc(None, target_bir_lowering=False, debug=False)
    d_attr = nc.dram_tensor("attr_tour", [P, TOUR_F], F32, kind="ExternalInput")
    d_lseq = nc.dram_tensor("levseq", [P, TOUR_F + 1], F32, kind="ExternalInput")
    d_thr = nc.dram_tensor("thr", [1, 1], F32, kind="ExternalInput")
    d_tri = nc.dram_tensor("tri", [P, P], F32, kind="ExternalInput")
    d_roff = nc.dram_tensor("roff", [P, 1], I32, kind="ExternalInput")
    d_woff = nc.dram_tensor("woff", [P, 1], I32, kind="ExternalInput")
    d_sidx = nc.dram_tensor("sidx", [P, SPW], I16, kind="ExternalInput")
    # f16 output: max rel quantization 2^-11, far under the 2e-2 gate;
    # halves the output write + host transfer
    d_y = nc.dram_tensor("y", [P, PIX_F], F16, kind="ExternalOutput")

    TAILF = (rlp + P - 1) // P + 1
    VNF = TOUR_F + TAILF

    with tile.TileContext(nc) as tc:
        dbufs = 2 if reps > 1 else 1
        with tc.tile_pool(name="dram", bufs=1, space="DRAM") as dpool, \
             tc.tile_pool(name="persist", bufs=1) as pp, \
             tc.tile_pool(name="work", bufs=dbufs) as wp, \
             tc.tile_pool(name="io", bufs=dbufs) as iop, \
             tc.tile_pool(name="psum", bufs=dbufs, space="PSUM") as sp:
            vflat = dpool.tile([P * VNF, 1], F16)

            # persistent constants
            t_ones = pp.tile([P, TOUR_F], F32)
            nc.vector.memset(t_ones[:], 1.0)
            t_tri = pp.tile([P, P], F32)
            nc.sync.dma_start(out=t_tri[:], in_=d_tri[:])
            t_thr = pp.tile([P, 1], F32)
            nc.sync.dma_start(out=t_thr[:], in_=d_thr[:].to_broadcast([P, 1]))
            t_thrb = pp.tile([P, 1], F32)
            nc.vector.tensor_scalar_mul(t_thrb[:], t_thr[:], -1000.0)
            if partial_write:
                t_woff = pp.tile([P, 1], I32)
                nc.sync.dma_start(out=t_woff[:], in_=d_woff[:])
            # zero-fill the window-overhang tail past position 2N
            t_tz = pp.tile([P, TAILF], F16)
            nc.vector.memset(t_tz[:], 0.0)
            nc.sync.dma_start(out=vflat[TWO_N:P * VNF, 0:1], in_=t_tz[:])

            for r in range(reps):
                t_attr = iop.tile([P, TOUR_F], F32, tag="attr")
                nc.sync.dma_start(out=t_attr[:], in_=d_attr[:])
                t_lseq = iop.tile([P, TOUR_F + 1], F32, tag="lseq")
                nc.sync.dma_start(out=t_lseq[:], in_=d_lseq[:])

                # sigma = sigmoid(1000*attr - 1000*thr)   (unclamped; the
                # +-12 clamp only changes sigma by <7e-6)
                nc.scalar.activation(
                    out=t_attr[:], in_=t_attr[:],
                    func=mybir.ActivationFunctionType.Sigmoid,
                    bias=t_thrb[:, :1], scale=1000.0)

                # e = sigma * (levseq[t] - levseq[t-1]); totals = row sums
                t_e = wp.tile([P, TOUR_F], F32, tag="e")
                nc.vector.tensor_sub(out=t_e[:], in0=t_lseq[:, 1:TOUR_F + 1],
                                     in1=t_lseq[:, 0:TOUR_F])
                t_tot = wp.tile([P, 1], F32, tag="tot")
                nc.vector.scalar_tensor_tensor(
                    out=t_e[:], in0=t_e[:], scalar=0.0, in1=t_attr[:],
                    op0=mybir.AluOpType.bypass, op1=mybir.AluOpType.mult,
                    accum_out=t_tot[:])

                # cross-partition exclusive prefix of totals (strict lower
                # triangular ones matmul), used as the scan's initial state
                t_cpsum = sp.tile([P, 1], F32, tag="carry")
                nc.tensor.matmul(t_cpsum[:], t_tri[:], t_tot[:])
                t_carry = wp.tile([P, 1], F32, tag="carrys")
                nc.scalar.copy(out=t_carry[:], in_=t_cpsum[:])

                t_ps = wp.tile([P, TOUR_F], F16, tag="ps")
                nc.vector.tensor_tensor_scan(
                    out=t_ps[:], data0=t_ones[:], data1=t_e[:],
                    initial=t_carry[:, :1],
                    op0=mybir.AluOpType.mult, op1=mybir.AluOpType.add)
                if partial_write:
                    # only the chunks this core's pixel windows read
                    # (permuted into partitions [0, NW)) take the round-trip
                    nc.gpsimd.indirect_dma_start(
                        out=vflat[:], out_offset=bass.IndirectOffsetOnAxis(
                            ap=t_woff[0:NW, 0:1], axis=0),
                        in_=t_ps[0:NW, :], in_offset=None)
                else:
                    nc.sync.dma_start(out=vflat[0:TWO_N, 0:1], in_=t_ps[:])

                # ---- pixel phase ----
                t_roff = wp.tile([P, 1], I32, tag="roff")
                nc.sync.dma_start(out=t_roff[:], in_=d_roff[:])
                t_sidx = iop.tile([P, SPW], I16, tag="sidx")
                nc.sync.dma_start(out=t_sidx[:], in_=d_sidx[:])

                t_pr = wp.tile([P, rlp], F16, tag="pr")
                nc.gpsimd.indirect_dma_start(
                    out=t_pr[:], out_offset=None, in_=vflat[:],
                    in_offset=bass.IndirectOffsetOnAxis(ap=t_roff[:, 0:1],
                                                        axis=0))
                t_pb = wp.tile([P, PIX_F], I16, tag="pb")
                col = 0
                for (s0, w, w0, w1) in segs:
                    nw = w1 - w0
                    nc.gpsimd.local_scatter(
                        out_ap=t_pb[:, s0:s0 + w],
                        data_ap=t_pr[:, w0:w1].bitcast(I16),
                        idxs_ap=t_sidx[:, col:col + nw],
                        channels=P, num_elems=w, num_idxs=nw)
                    col += nw
                # mask = (pb == 0): 1.0 inside runs (keep state), 0.0 at
                # run starts (reset to the scattered value)
                t_am = wp.tile([P, PIX_F], F16, tag="am")
                nc.vector.tensor_single_scalar(
                    out=t_am[:], in_=t_pb[:].bitcast(F16), scalar=0.0,
                    op=mybir.AluOpType.is_equal)
                t_y = wp.tile([P, PIX_F], F16, tag="y")
                nc.vector.tensor_tensor_scan(
                    out=t_y[:], data0=t_am[:], data1=t_pb[:].bitcast(F16),
                    initial=0.0, op0=mybir.AluOpType.mult,
                    op1=mybir.AluOpType.add)
                nc.sync.dma_start(out=d_y[:], in_=t_y[:])
    nc.finalize()
    return nc


# ======================= orchestration =======================

def build_all(attr, levels, parent, p2n):
    """All host-side metadata + per-core input maps (minus thr/tri)."""
    per_tree = []
    for t in range(T):
        entry, exit_ = build_tour(parent[t])
        at, ls = build_tree_tensors(attr[t], levels[t], parent[t], entry, exit_)
        per_tree.append(dict(entry=entry, attr_tour=at, levseq=ls))

    srcpos_by_core, my_by_core = [], []
    for t in range(T):
        srcpos = per_tree[t]["entry"][p2n[t].astype(np.int64)]
        ordx = np.argsort(srcpos, kind="stable")
        for half in range(2):
            my = ordx[half * PIX_PER_CORE:(half + 1) * PIX_PER_CORE]
            my_by_core.append(my)
            srcpos_by_core.append(srcpos[my])
    pix = build_pixel_meta(srcpos_by_core)
    pix["my"] = my_by_core
    pix["per_tree"] = per_tree

    # ---- chunk permutation: each core's pixel windows only touch ~half of
    # the tour, so only those prefix chunks need the DRAM round-trip.  The
    # write slice [0:NW) is a shared program immediate, so per core we
    # permute tour chunks across partitions to put the needed chunks first;
    # the carry matmul's triangular matrix is permuted to match and the
    # write destinations come from a per-core offset tensor. ----
    ranges = []
    for c in pix["cores"]:
        ro = c["roff"].ravel().astype(np.int64)
        q0 = int(ro.min()) // TOUR_F
        q1 = min(P, -(-(int(ro.max()) + pix["rlp"]) // TOUR_F))
        ranges.append((q0, q1))
    NW = max(q1 - q0 for (q0, q1) in ranges)
    pix["NW"] = NW
    for ci, c in enumerate(pix["cores"]):
        q0, q1 = ranges[ci]
        need = list(range(q0, q1))
        rest = [q for q in range(P) if q < q0 or q >= q1]
        pad = rest[:NW - len(need)]
        tail = rest[NW - len(need):]
        chunk = np.array(need + pad + tail, np.int64)   # chunk_of_partition
        assert chunk.size == P and np.array_equal(np.sort(chunk), np.arange(P))
        c["chunk"] = chunk
        c["woff"] = (chunk * TOUR_F).astype(np.int32).reshape(P, 1)
        # tri[k, m] = 1 iff chunk[k] < chunk[m]
        c["tri"] = (chunk[:, None] < chunk[None, :]).astype(np.float32)
    return pix


def make_in_maps(pix, thr, perm=False):
    """perm=True pairs with build_bass(partial_write=True): tour chunks are
    permuted per core so the needed prefix chunks sit in partitions [0,NW)."""
    thr2 = np.asarray(thr, np.float32).reshape(1, 1)
    ident = np.arange(P, dtype=np.int64)
    tri_std = (ident[:, None] < ident[None, :]).astype(np.float32)
    in_maps = []
    for ci in range(8):
        t = ci // 2
        c = pix["cores"][ci]
        chunk = c["chunk"] if perm else ident
        at = pix["per_tree"][t]["attr_tour"][chunk]
        ls = pix["per_tree"][t]["levseq"][chunk]
        in_maps.append(dict(
            attr_tour=np.ascontiguousarray(at),
            levseq=np.ascontiguousarray(ls),
            thr=thr2, tri=c["tri"] if perm else tri_std, roff=c["roff"],
            woff=(chunk * TOUR_F).astype(np.int32).reshape(P, 1),
            sidx=c["sidx"]))
    return in_maps


def build_all_chunked(attr, levels, parent, p2n):
    per_tree = []
    srcpos_by_tree = []
    for t in range(T):
        entry, exit_ = build_tour(parent[t])
        at, ls = build_tree_tensors(attr[t], levels[t], parent[t], entry, exit_)
        per_tree.append(dict(attr_tour=at, levseq=ls))
        srcpos_by_tree.append(entry[p2n[t].astype(np.int64)])
    pixc = build_pixel_meta_chunked(srcpos_by_tree)
    pixc["per_tree"] = per_tree
    return pixc


def make_in_maps_chunked(pixc, thr):
    thr2 = np.asarray(thr, np.float32).reshape(1, 1)
    ident = np.arange(P, dtype=np.int64)
    tri = (ident[:, None] < ident[None, :]).astype(np.float32)
    in_maps = []
    for ci in range(8):
        t = ci // 2
        in_maps.append(dict(
            attr_tour=pixc["per_tree"][t]["attr_tour"],
            levseq=pixc["per_tree"][t]["levseq"],
            thr=thr2, tri=tri, sidx=pixc["cores"][ci]["sidx"]))
    return in_maps


def kernel(**inputs):
    x = np.asarray(inputs["x"])
    attr = np.asarray(inputs["attr_norm"], dtype=np.float32)
    levels = np.asarray(inputs["levels"], dtype=np.float32)
    thr = np.asarray(inputs["thr"], dtype=np.float32)
    parent = np.asarray(inputs["parent"], dtype=np.int32)
    p2n = np.asarray(inputs["pixel_to_node"], dtype=np.int32)
    B, Cc, H, W = x.shape

    # The sorted-rank layout with a DRAM round-trip of the f16 prefix
    # measured fastest (33us/call); the chunked SBUF-direct variant
    # (build_*_chunked) loses on scatter-index size (48.5us) and is kept
    # only as a documented experiment.
    pix = build_all(attr.reshape(T, -1), levels.reshape(T, -1),
                    parent.reshape(T, -1), p2n.reshape(T, -1))
    nc = build_bass(pix)
    in_maps = make_in_maps(pix, thr)
    res = run_bass_kernel_spmd(nc, in_maps, list(range(8)))

    y = np.zeros((T, H * W), np.float32)
    for ci in range(8):
        t = ci // 2
        y[t][pix["my"][ci]] = res.results[ci]["y"].ravel()
    return y.reshape(B, Cc, H, W)


# revision 43
# speedup vs baseline: 1.5364x; 1.0301x over previous
"""Euler-tour connected-filter kernel for TRN2 (8 cores, data-parallel).

Math: v[i] = levels[root] + sum over root->i path of sigma_j * delta_j.
Place +sigma*delta at the tour slot where a node is entered and
-sigma*delta where it is exited; v[i] is then the inclusive prefix sum of
that 2N-long sequence at entry(i).  The whole 32-deep level-by-level
propagation collapses into one per-partition scan plus a 128-wide
cross-partition carry (triangular matmul).

Key packing trick: store lev[node] at entry slots and lev[parent] at exit
slots ("levseq").  Then for EVERY tour slot t:
    e[t] = sigma(attr_tour[t]) * (levseq[t] - levseq[t-1])
which is +sigma*delta at entries and exactly -sigma*delta at exits (the
subtraction is the exact IEEE negation, so closed subtrees cancel to the
rounding of the running sum).  The device therefore needs only TWO tour
arrays.  levseq is shipped as a [128, 4097] sliding view so the t-1 shift
never crosses a partition boundary; the virtual levseq[-1] is 0 and
attr_tour[0]=2.0 makes sigma=1 exactly, so slot 0 contributes
levels[root] like the reference's root override.

Host work is index arithmetic / layout only (depths, subtree sizes, tour
positions, sorting, gathers); every float op of the reference runs on
device.

Pixel phase: per core 524288 pixels sorted by source tour position; per
partition a contiguous window of the (f16) prefix array is fetched by
indirect DMA, run-start values are placed by gpsimd local_scatter, the
run mask is derived on device as (pb == 0) (real prefix values are never
0.0 since v >= levels[root] > 0.1), and a masked f16 scan expands runs to
per-pixel values; host unpermutes.  Output is f16 (max quantization 2^-11,
vs the 2e-2 correctness gate); measured end-to-end rel err 5.1e-4.

Measured on 8 axon-tunneled TRN2 cores (all 8 running concurrently):
~33us sustained on-device time per call, via interleaved marginal cost of
in-NEFF repetitions.  HBM traffic is ~7.25MB/core/call (attr_tour 2MB +
levseq 2MB + sidx 0.63MB + prefix write 1MB + window read 0.55MB + y 1MB),
i.e. ~220GB/s/core sustained under 8-core contention -- memory-bound, as
the target regime intends.  The earlier BFS level-expand formulation moved
~18MB/core through a 32-level serial chain and needed 10.3MB of inputs.
"""
import numpy as np

P = 128
N = 262144
TWO_N = 2 * N
TOUR_F = TWO_N // P          # 4096
PIX_PER_CORE = 524288
PIX_F = PIX_PER_CORE // P    # 4096
SEG = 2046                   # local_scatter num_elems limit (int16 units)
T = 4


# ======================= host: tour construction =======================

def build_tour(par):
    """entry/exit tour positions for one tree (children in node-id order)."""
    par = par.astype(np.int64)
    # depth via pointer doubling
    anc = par.copy(); anc[0] = N
    dep = np.ones(N, np.int64); dep[0] = 0
    anc_e = np.concatenate([anc, [N]])
    dep_e = np.concatenate([dep, [0]])
    while (anc_e[:N] != N).any():
        dep_e = dep_e + dep_e[anc_e]
        anc_e = anc_e[anc_e]
    depth = dep_e[:N]
    D = int(depth.max())
    order_by_depth = np.argsort(depth, kind="stable")
    counts = np.bincount(depth, minlength=D + 1)
    splits = np.split(order_by_depth, np.cumsum(counts)[:-1])

    # subtree sizes, deepest level first
    size = np.ones(N, np.int64)
    for dd in range(D, 0, -1):
        nd = splits[dd]
        np.add.at(size, par[nd], size[nd])

    # within-parent exclusive cumsum of sibling subtree sizes
    ch_order = np.argsort(par[1:], kind="stable") + 1
    pp = par[ch_order]
    sz = size[ch_order]
    cs = np.cumsum(sz) - sz
    starts = np.concatenate([[True], pp[1:] != pp[:-1]])
    start_cs = np.maximum.accumulate(np.where(starts, cs, -1))
    childoff = np.empty(N, np.int64)
    childoff[ch_order] = cs - start_cs
    childoff[0] = 0

    entry = np.zeros(N, np.int64)
    for dd in range(1, D + 1):
        nd = splits[dd]
        entry[nd] = entry[par[nd]] + 1 + 2 * childoff[nd]
    exit_ = entry + 2 * size - 1
    return entry, exit_


def build_tree_tensors(attr_t, lev_t, par, entry, exit_):
    """attr_tour [P, TOUR_F] and levseq [P, TOUR_F+1] device inputs."""
    attr_tour = np.empty(TWO_N, np.float32)
    levflat = np.empty(TWO_N, np.float32)
    attr_tour[entry] = attr_t
    attr_tour[exit_] = attr_t
    levflat[entry] = lev_t
    levflat[exit_] = lev_t[par.astype(np.int64)]
    attr_tour[0] = 2.0   # root: sigma(1000*(2-thr)) == 1.0 exactly
    arr2 = np.concatenate([np.zeros(1, np.float32), levflat])
    levseq = np.lib.stride_tricks.sliding_window_view(
        arr2, TOUR_F + 1)[::TOUR_F].copy()
    return attr_tour.reshape(P, TOUR_F).copy(), levseq


# ======================= host: pixel metadata =======================

def build_pixel_meta(srcpos_sorted_by_core):
    """Uniform (across 8 cores) window/segment layout + per-core scatter
    indices and masks.  srcpos_sorted_by_core: 8 arrays [PIX_PER_CORE]."""
    sp = [s.reshape(P, PIX_F) for s in srcpos_sorted_by_core]
    nlo = [s[:, 0].astype(np.int32) for s in sp]
    span = max(int((s[:, -1] - s[:, 0]).max()) for s in sp)

    runs = []
    for s, lo in zip(sp, nlo):
        per = []
        for p in range(P):
            row = s[p]
            st = np.flatnonzero(np.concatenate([[True], row[1:] != row[:-1]]))
            per.append(((row[st] - lo[p]).astype(np.int64), st.astype(np.int64)))
        runs.append(per)

    # prefix values are f16 in the window, so one scatter index per value
    # (no int16-pair splitting); dst segments over the 4096 pixel slots
    segs = []
    s0 = 0
    while s0 < PIX_F:
        w = min(SEG, PIX_F - s0)
        w -= w % 2
        segs.append((s0, w))
        s0 += w

    seg_meta = []
    for (s0, w) in segs:
        f0, f1 = s0, s0 + w
        w0g, w1g = 1 << 30, 0
        sel = []
        for per in runs:
            selc = []
            for p in range(P):
                m, ell = per[p]
                k = (ell >= f0) & (ell < f1)
                mm, ee = m[k], ell[k]
                selc.append((mm, ee))
                if mm.size:
                    w0g = min(w0g, int(mm.min()))
                    w1g = max(w1g, int(mm.max()) + 1)
            sel.append(selc)
        if w0g >= w1g:
            w0g, w1g = 0, 2
        w1g += (w1g - w0g) % 2   # even num_idxs
        seg_meta.append(dict(s0=s0, w=w, w0=w0g, w1=w1g, sel=sel))

    rlp = max(span + 2, max(sm["w1"] for sm in seg_meta) + 2)
    SPW = sum(sm["w1"] - sm["w0"] for sm in seg_meta)

    # no mask tensor: the device derives it as (pb == 0) — local_scatter
    # zeroes unwritten slots and real prefix values are never 0.0
    cores = []
    for ci in range(8):
        parts = []
        for sm in seg_meta:
            s0, w0, w1 = sm["s0"], sm["w0"], sm["w1"]
            idx = np.full((P, w1 - w0), -1, np.int16)
            for p in range(P):
                mm, ee = sm["sel"][ci][p]
                idx[p, mm - w0] = (ee - s0).astype(np.int16)
            parts.append(idx)
        cores.append(dict(sidx=np.concatenate(parts, axis=1),
                          roff=nlo[ci].reshape(P, 1).astype(np.int32)))
    return dict(rlp=rlp, SPW=SPW,
                segs=[(sm["s0"], sm["w"], sm["w0"], sm["w1"])
                      for sm in seg_meta],
                cores=cores)


# ======================= host: chunked pixel metadata =======================

def build_pixel_meta_chunked(srcpos_by_tree):
    """Pixels assigned to partition p = srcpos // TOUR_F (their tour chunk),
    so the scatter source is the scan output t_ps[p, :] directly in SBUF --
    no DRAM round-trip of the prefix, no indirect window gather.  Each core
    takes half of every chunk's (sorted) pixels; rows are padded to the max
    per-partition count C.  Returns per-core sidx plus the pixel-id lists
    needed to unpermute on the host."""
    percore = []   # 8 entries: list of P arrays of pixel ids (sorted)
    for t in range(T):
        srcpos = srcpos_by_tree[t]
        ordx = np.argsort(srcpos, kind="stable")
        ssp = srcpos[ordx]
        bounds = np.searchsorted(ssp, np.arange(P + 1) * TOUR_F)
        rows0, rows1 = [], []
        for p in range(P):
            lo, hi = int(bounds[p]), int(bounds[p + 1])
            mid = lo + ((hi - lo) + 1) // 2
            rows0.append(ordx[lo:mid])
            rows1.append(ordx[mid:hi])
        percore.append(rows0)
        percore.append(rows1)

    maxn = max(max(r.size for r in rows) for rows in percore)
    CG = maxn + 128   # placement target scale; C is finalized below

    # runs per (core, partition): m = srcpos - p*TOUR_F.  Run start slots are
    # placed PROPORTIONALLY (ell ~ m*CG/TOUR_F, pushed right on collision via
    # a running-max recurrence) so a destination segment always maps to a
    # narrow source window regardless of per-row pixel-count variation.
    # Gap slots between runs inherit the previous run's value in the scan
    # (pb==0 -> mask 1) and are simply never read back by the host.
    runs = []      # per core: per partition (m_run, ell_run)
    ells = []      # per core: per partition ell_of_pixel
    C = 0
    for ci in range(8):
        t = ci // 2
        srcpos = srcpos_by_tree[t]
        per, pere = [], []
        for p in range(P):
            row = srcpos[percore[ci][p]]
            n = row.size
            if n == 0:
                per.append((np.zeros(0, np.int64), np.zeros(0, np.int64)))
                pere.append(np.zeros(0, np.int64))
                continue
            st = np.flatnonzero(np.concatenate([[True], row[1:] != row[:-1]]))
            m = (row[st] - p * TOUR_F).astype(np.int64)
            rl = np.diff(np.concatenate([st, [n]]))
            cumprev = np.concatenate([[0], np.cumsum(rl)[:-1]])
            # proportional target, capped so the packed remainder always
            # fits in CG slots (=> end <= CG for every row)
            target = np.minimum((m * CG) // TOUR_F, CG - (n - cumprev))
            b = np.maximum.accumulate(target - cumprev)
            ell = b + cumprev
            C = max(C, int(ell[-1] + rl[-1]))
            per.append((m, ell))
            pere.append(np.repeat(ell - cumprev, rl) + np.arange(n))
        runs.append(per)
        ells.append(pere)
    C += C % 2

    segs = []
    s0 = 0
    while s0 < C:
        w = min(SEG, C - s0)
        w -= w % 2
        segs.append((s0, w))
        s0 += w

    seg_meta = []
    for (s0, w) in segs:
        f0, f1 = s0, s0 + w
        w0g, w1g = 1 << 30, 0
        for per in runs:
            for p in range(P):
                m, ell = per[p]
                k = (ell >= f0) & (ell < f1)
                if k.any():
                    mm = m[k]
                    w0g = min(w0g, int(mm.min()))
                    w1g = max(w1g, int(mm.max()) + 1)
        if w0g >= w1g:
            w0g, w1g = 0, 2
        w1g = min(TOUR_F, w1g + (w1g - w0g) % 2)
        if (w1g - w0g) % 2:
            w0g -= 1
        seg_meta.append(dict(s0=s0, w=w, w0=w0g, w1=w1g))

    SPW = sum(sm["w1"] - sm["w0"] for sm in seg_meta)
    cores = []
    for ci in range(8):
        parts = []
        for sm in seg_meta:
            s0, w0, w1 = sm["s0"], sm["w0"], sm["w1"]
            f0, f1 = s0, s0 + sm["w"]
            idx = np.full((P, w1 - w0), -1, np.int16)
            for p in range(P):
                m, ell = runs[ci][p]
                k = (ell >= f0) & (ell < f1)
                idx[p, m[k] - w0] = (ell[k] - s0).astype(np.int16)
            parts.append(idx)
        cores.append(dict(sidx=np.concatenate(parts, axis=1)))
    return dict(C=C, SPW=SPW,
                segs=[(sm["s0"], sm["w"], sm["w0"], sm["w1"])
                      for sm in seg_meta],
                cores=cores, rows=percore, ells=ells)


# ======================= device program =======================
import sys
if '/opt/trn_rl_repo' not in sys.path:
    sys.path.insert(0, '/opt/trn_rl_repo')
from concourse import bass, mybir, tile, bacc
from concourse.bass_utils import run_bass_kernel_spmd

F32 = mybir.dt.float32
F16 = mybir.dt.float16
I32 = mybir.dt.int32
I16 = mybir.dt.int16


def build_bass_chunked(pixc, reps=1):
    """Chunk-partitioned pixel phase: scatter straight from the scan output
    in SBUF; no prefix DRAM round-trip, no indirect DMA anywhere."""
    C = pixc["C"]; SPW = pixc["SPW"]; segs = pixc["segs"]

    nc = bacc.Bacc(None, target_bir_lowering=False, debug=False)
    d_attr = nc.dram_tensor("attr_tour", [P, TOUR_F], F32, kind="ExternalInput")
    d_lseq = nc.dram_tensor("levseq", [P, TOUR_F + 1], F32, kind="ExternalInput")
    d_thr = nc.dram_tensor("thr", [1, 1], F32, kind="ExternalInput")
    d_tri = nc.dram_tensor("tri", [P, P], F32, kind="ExternalInput")
    d_sidx = nc.dram_tensor("sidx", [P, SPW], I16, kind="ExternalInput")
    d_y = nc.dram_tensor("y", [P, C], F16, kind="ExternalOutput")

    with tile.TileContext(nc) as tc:
        dbufs = 2 if reps > 1 else 1
        with tc.tile_pool(name="persist", bufs=1) as pp, \
             tc.tile_pool(name="work", bufs=1) as wp, \
             tc.tile_pool(name="hot", bufs=dbufs) as hp, \
             tc.tile_pool(name="io", bufs=dbufs) as iop, \
             tc.tile_pool(name="psum", bufs=dbufs, space="PSUM") as sp:
            t_ones = pp.tile([P, TOUR_F], F16)
            nc.vector.memset(t_ones[:], 1.0)
            t_tri = pp.tile([P, P], F32)
            nc.sync.dma_start(out=t_tri[:], in_=d_tri[:])
            t_thr = pp.tile([P, 1], F32)
            nc.sync.dma_start(out=t_thr[:], in_=d_thr[:].to_broadcast([P, 1]))
            t_thrb = pp.tile([P, 1], F32)
            nc.vector.tensor_scalar_mul(t_thrb[:], t_thr[:], -1000.0)

            for r in range(reps):
                t_attr = iop.tile([P, TOUR_F], F32, tag="attr")
                nc.sync.dma_start(out=t_attr[:], in_=d_attr[:])
                t_lseq = iop.tile([P, TOUR_F + 1], F32, tag="lseq")
                nc.sync.dma_start(out=t_lseq[:], in_=d_lseq[:])
                t_sidx = iop.tile([P, SPW], I16, tag="sidx")
                nc.sync.dma_start(out=t_sidx[:], in_=d_sidx[:])

                nc.scalar.activation(
                    out=t_attr[:], in_=t_attr[:],
                    func=mybir.ActivationFunctionType.Sigmoid,
                    bias=t_thrb[:, :1], scale=1000.0)
                t_e = wp.tile([P, TOUR_F], F32, tag="e")
                nc.vector.tensor_sub(out=t_e[:], in0=t_lseq[:, 1:TOUR_F + 1],
                                     in1=t_lseq[:, 0:TOUR_F])
                t_tot = wp.tile([P, 1], F32, tag="tot")
                nc.vector.scalar_tensor_tensor(
                    out=t_e[:], in0=t_e[:], scalar=0.0, in1=t_attr[:],
                    op0=mybir.AluOpType.bypass, op1=mybir.AluOpType.mult,
                    accum_out=t_tot[:])
                t_cpsum = sp.tile([P, 1], F32, tag="carry")
                nc.tensor.matmul(t_cpsum[:], t_tri[:], t_tot[:])
                t_carry = wp.tile([P, 1], F32, tag="carrys")
                nc.scalar.copy(out=t_carry[:], in_=t_cpsum[:])
                t_ps = hp.tile([P, TOUR_F], F16, tag="ps")
                nc.vector.tensor_tensor_scan(
                    out=t_ps[:], data0=t_ones[:], data1=t_e[:],
                    initial=t_carry[:, :1],
                    op0=mybir.AluOpType.mult, op1=mybir.AluOpType.add)

                # pixel phase straight from SBUF
                t_pb = wp.tile([P, C], I16, tag="pb")
                col = 0
                for (s0, w, w0, w1) in segs:
                    nw = w1 - w0
                    nc.gpsimd.local_scatter(
                        out_ap=t_pb[:, s0:s0 + w],
                        data_ap=t_ps[:, w0:w1].bitcast(I16),
                        idxs_ap=t_sidx[:, col:col + nw],
                        channels=P, num_elems=w, num_idxs=nw)
                    col += nw
                t_am = wp.tile([P, C], F16, tag="am")
                nc.vector.tensor_single_scalar(
                    out=t_am[:], in_=t_pb[:].bitcast(F16), scalar=0.0,
                    op=mybir.AluOpType.is_equal)
                t_y = hp.tile([P, C], F16, tag="y")
                nc.vector.tensor_tensor_scan(
                    out=t_y[:], data0=t_am[:], data1=t_pb[:].bitcast(F16),
                    initial=0.0, op0=mybir.AluOpType.mult,
                    op1=mybir.AluOpType.add)
                nc.sync.dma_start(out=d_y[:], in_=t_y[:])
    nc.finalize()
    return nc


def build_bass(pix, reps=1, partial_write=False):
    rlp = pix["rlp"]; SPW = pix["SPW"]; segs = pix["segs"]; NW = pix["NW"]

    nc = bacc.Bacc(None, target_bir_lowering=False, debug=False)
    d_attr = nc.dram_tensor("attr_tour", [P, TOUR_F], F32, kind="ExternalInput")
    d_lseq = nc.dram_tensor("levseq", [P, TOUR_F + 1], F32, kind="ExternalInput")
    d_thr = nc.dram_tensor("thr", [1, 1], F32, kind="ExternalInput")
    d_tri = nc.dram_tensor("tri", [P, P], F32, kind="ExternalInput")
    d_roff = nc.dram_tensor("roff", [P, 1], I32, kind="ExternalInput")
    d_woff = nc.dram_tensor("woff", [P, 1], I32, kind="ExternalInput")
    d_sidx = nc.dram_tensor("sidx", [P, SPW], I16, kind="ExternalInput")
    # f16 output: max rel quantization 2^-11, far under the 2e-2 gate;
    # halves the output write + host transfer
    d_y = nc.dram_tensor("y", [P, PIX_F], F16, kind="ExternalOutput")

    TAILF = (rlp + P - 1) // P + 1
    VNF = TOUR_F + TAILF

    with tile.TileContext(nc) as tc:
        dbufs = 2 if reps > 1 else 1
        with tc.tile_pool(name="dram", bufs=1, space="DRAM") as dpool, \
             tc.tile_pool(name="persist", bufs=1) as pp, \
             tc.tile_pool(name="work", bufs=dbufs) as wp, \
             tc.tile_pool(name="io", bufs=dbufs) as iop, \
             tc.tile_pool(name="psum", bufs=dbufs, space="PSUM") as sp:
            vflat = dpool.tile([P * VNF, 1], F16)

            # persistent constants
            t_ones = pp.tile([P, TOUR_F], F32)
            nc.vector.memset(t_ones[:], 1.0)
            t_tri = pp.tile([P, P], F32)
            nc.sync.dma_start(out=t_tri[:], in_=d_tri[:])
            t_thr = pp.tile([P, 1], F32)
            nc.sync.dma_start(out=t_thr[:], in_=d_thr[:].to_broadcast([P, 1]))
            t_thrb = pp.tile([P, 1], F32)
            nc.vector.tensor_scalar_mul(t_thrb[:], t_thr[:], -1000.0)
            if partial_write:
                t_woff = pp.tile([P, 1], I32)
                nc.sync.dma_start(out=t_woff[:], in_=d_woff[:])
            # zero-fill the window-overhang tail past position 2N
            t_tz = pp.tile([P, TAILF], F16)
            nc.vector.memset(t_tz[:], 0.0)
            nc.sync.dma_start(out=vflat[TWO_N:P * VNF, 0:1], in_=t_tz[:])

            for r in range(reps):
                # spread the big transfers across per-engine DMA queues
                # (queue = issuing engine): sync alone serializes ~20us of
                # traffic on one ring
                t_attr = iop.tile([P, TOUR_F], F32, tag="attr")
                nc.scalar.dma_start(out=t_attr[:], in_=d_attr[:])
                t_lseq = iop.tile([P, TOUR_F + 1], F32, tag="lseq")
                nc.gpsimd.dma_start(out=t_lseq[:], in_=d_lseq[:])

                # sigma = sigmoid(1000*attr - 1000*thr)   (unclamped; the
                # +-12 clamp only changes sigma by <7e-6)
                nc.scalar.activation(
                    out=t_attr[:], in_=t_attr[:],
                    func=mybir.ActivationFunctionType.Sigmoid,
                    bias=t_thrb[:, :1], scale=1000.0)

                # e = sigma * (levseq[t] - levseq[t-1]); totals = row sums
                t_e = wp.tile([P, TOUR_F], F32, tag="e")
                nc.vector.tensor_sub(out=t_e[:], in0=t_lseq[:, 1:TOUR_F + 1],
                                     in1=t_lseq[:, 0:TOUR_F])
                t_tot = wp.tile([P, 1], F32, tag="tot")
                nc.vector.scalar_tensor_tensor(
                    out=t_e[:], in0=t_e[:], scalar=0.0, in1=t_attr[:],
                    op0=mybir.AluOpType.bypass, op1=mybir.AluOpType.mult,
                    accum_out=t_tot[:])

                # cross-partition exclusive prefix of totals (strict lower
                # triangular ones matmul), used as the scan's initial state
                t_cpsum = sp.tile([P, 1], F32, tag="carry")
                nc.tensor.matmul(t_cpsum[:], t_tri[:], t_tot[:])
                t_carry = wp.tile([P, 1], F32, tag="carrys")
                nc.scalar.copy(out=t_carry[:], in_=t_cpsum[:])

                t_ps = wp.tile([P, TOUR_F], F16, tag="ps")
                nc.vector.tensor_tensor_scan(
                    out=t_ps[:], data0=t_ones[:], data1=t_e[:],
                    initial=t_carry[:, :1],
                    op0=mybir.AluOpType.mult, op1=mybir.AluOpType.add)
                if partial_write:
                    # only the chunks this core's pixel windows read
                    # (permuted into partitions [0, NW)) take the round-trip
                    nc.gpsimd.indirect_dma_start(
                        out=vflat[:], out_offset=bass.IndirectOffsetOnAxis(
                            ap=t_woff[0:NW, 0:1], axis=0),
                        in_=t_ps[0:NW, :], in_offset=None)
                else:
                    nc.sync.dma_start(out=vflat[0:TWO_N, 0:1], in_=t_ps[:])

                # ---- pixel phase ----
                t_roff = wp.tile([P, 1], I32, tag="roff")
                nc.sync.dma_start(out=t_roff[:], in_=d_roff[:])
                t_sidx = iop.tile([P, SPW], I16, tag="sidx")
                nc.sync.dma_start(out=t_sidx[:], in_=d_sidx[:])

                t_pr = wp.tile([P, rlp], F16, tag="pr")
                nc.gpsimd.indirect_dma_start(
                    out=t_pr[:], out_offset=None, in_=vflat[:],
                    in_offset=bass.IndirectOffsetOnAxis(ap=t_roff[:, 0:1],
                                                        axis=0))
                t_pb = wp.tile([P, PIX_F], I16, tag="pb")
                col = 0
                for (s0, w, w0, w1) in segs:
                    nw = w1 - w0
                    nc.gpsimd.local_scatter(
                        out_ap=t_pb[:, s0:s0 + w],
                        data_ap=t_pr[:, w0:w1].bitcast(I16),
                        idxs_ap=t_sidx[:, col:col + nw],
                        channels=P, num_elems=w, num_idxs=nw)
                    col += nw
                # mask = (pb == 0): 1.0 inside runs (keep state), 0.0 at
                # run starts (reset to the scattered value)
                t_am = wp.tile([P, PIX_F], F16, tag="am")
                nc.vector.tensor_single_scalar(
                    out=t_am[:], in_=t_pb[:].bitcast(F16), scalar=0.0,
                    op=mybir.AluOpType.is_equal)
                t_y = wp.tile([P, PIX_F], F16, tag="y")
                nc.vector.tensor_tensor_scan(
                    out=t_y[:], data0=t_am[:], data1=t_pb[:].bitcast(F16),
                    initial=0.0, op0=mybir.AluOpType.mult,
                    op1=mybir.AluOpType.add)
                nc.sync.dma_start(out=d_y[:], in_=t_y[:])
    nc.finalize()
    return nc


# ======================= orchestration =======================

def build_all(attr, levels, parent, p2n):
    """All host-side metadata + per-core input maps (minus thr/tri)."""
    per_tree = []
    for t in range(T):
        entry, exit_ = build_tour(parent[t])
        at, ls = build_tree_tensors(attr[t], levels[t], parent[t], entry, exit_)
        per_tree.append(dict(entry=entry, attr_tour=at, levseq=ls))

    srcpos_by_core, my_by_core = [], []
    for t in range(T):
        srcpos = per_tree[t]["entry"][p2n[t].astype(np.int64)]
        ordx = np.argsort(srcpos, kind="stable")
        for half in range(2):
            my = ordx[half * PIX_PER_CORE:(half + 1) * PIX_PER_CORE]
            my_by_core.append(my)
            srcpos_by_core.append(srcpos[my])
    pix = build_pixel_meta(srcpos_by_core)
    pix["my"] = my_by_core
    pix["per_tree"] = per_tree

    # ---- chunk permutation: each core's pixel windows only touch ~half of
    # the tour, so only those prefix chunks need the DRAM round-trip.  The
    # write slice [0:NW) is a shared program immediate, so per core we
    # permute tour chunks across partitions to put the needed chunks first;
    # the carry matmul's triangular matrix is permuted to match and the
    # write destinations come from a per-core offset tensor. ----
    ranges = []
    for c in pix["cores"]:
        ro = c["roff"].ravel().astype(np.int64)
        q0 = int(ro.min()) // TOUR_F
        q1 = min(P, -(-(int(ro.max()) + pix["rlp"]) // TOUR_F))
        ranges.append((q0, q1))
    NW = max(q1 - q0 for (q0, q1) in ranges)
    pix["NW"] = NW
    for ci, c in enumerate(pix["cores"]):
        q0, q1 = ranges[ci]
        need = list(range(q0, q1))
        rest = [q for q in range(P) if q < q0 or q >= q1]
        pad = rest[:NW - len(need)]
        tail = rest[NW - len(need):]
        chunk = np.array(need + pad + tail, np.int64)   # chunk_of_partition
        assert chunk.size == P and np.array_equal(np.sort(chunk), np.arange(P))
        c["chunk"] = chunk
        c["woff"] = (chunk * TOUR_F).astype(np.int32).reshape(P, 1)
        # tri[k, m] = 1 iff chunk[k] < chunk[m]
        c["tri"] = (chunk[:, None] < chunk[None, :]).astype(np.float32)
    return pix


def make_in_maps(pix, thr, perm=False):
    """perm=True pairs with build_bass(partial_write=True): tour chunks are
    permuted per core so the needed prefix chunks sit in partitions [0,NW)."""
    thr2 = np.asarray(thr, np.float32).reshape(1, 1)
    ident = np.arange(P, dtype=np.int64)
    tri_std = (ident[:, None] < ident[None, :]).astype(np.float32)
    in_maps = []
    for ci in range(8):
        t = ci // 2
        c = pix["cores"][ci]
        chunk = c["chunk"] if perm else ident
        at = pix["per_tree"][t]["attr_tour"][chunk]
        ls = pix["per_tree"][t]["levseq"][chunk]
        in_maps.append(dict(
            attr_tour=np.ascontiguousarray(at),
            levseq=np.ascontiguousarray(ls),
            thr=thr2, tri=c["tri"] if perm else tri_std, roff=c["roff"],
            woff=(chunk * TOUR_F).astype(np.int32).reshape(P, 1),
            sidx=c["sidx"]))
    return in_maps


def build_all_chunked(attr, levels, parent, p2n):
    per_tree = []
    srcpos_by_tree = []
    for t in range(T):
        entry, exit_ = build_tour(parent[t])
        at, ls = build_tree_tensors(attr[t], levels[t], parent[t], entry, exit_)
        per_tree.append(dict(attr_tour=at, levseq=ls))
        srcpos_by_tree.append(entry[p2n[t].astype(np.int64)])
    pixc = build_pixel_meta_chunked(srcpos_by_tree)
    pixc["per_tree"] = per_tree
    return pixc


def make_in_maps_chunked(pixc, thr):
    thr2 = np.asarray(thr, np.float32).reshape(1, 1)
    ident = np.arange(P, dtype=np.int64)
    tri = (ident[:, None] < ident[None, :]).astype(np.float32)
    in_maps = []
    for ci in range(8):
        t = ci // 2
        in_maps.append(dict(
            attr_tour=pixc["per_tree"][t]["attr_tour"],
            levseq=pixc["per_tree"][t]["levseq"],
            thr=thr2, tri=tri, sidx=pixc["cores"][ci]["sidx"]))
    return in_maps


def kernel(**inputs):
    x = np.asarray(inputs["x"])
    attr = np.asarray(inputs["attr_norm"], dtype=np.float32)
    levels = np.asarray(inputs["levels"], dtype=np.float32)
    thr = np.asarray(inputs["thr"], dtype=np.float32)
    parent = np.asarray(inputs["parent"], dtype=np.int32)
    p2n = np.asarray(inputs["pixel_to_node"], dtype=np.int32)
    B, Cc, H, W = x.shape

    # The sorted-rank layout with a DRAM round-trip of the f16 prefix
    # measured fastest (33us/call); the chunked SBUF-direct variant
    # (build_*_chunked) loses on scatter-index size (48.5us) and is kept
    # only as a documented experiment.
    pix = build_all(attr.reshape(T, -1), levels.reshape(T, -1),
                    parent.reshape(T, -1), p2n.reshape(T, -1))
    nc = build_bass(pix)
    in_maps = make_in_maps(pix, thr)
    res = run_bass_kernel_spmd(nc, in_maps, list(range(8)))

    y = np.zeros((T, H * W), np.float32)
    for ci in range(8):
        t = ci // 2
        y[t][pix["my"][ci]] = res.results[ci]["y"].ravel()
    return y.reshape(B, Cc, H, W)
